# revision 1
# baseline (speedup 1.0000x reference)
"""HRM dense-transformer kernel for 8 trn2 NeuronCores.

Sharding: data-parallel over batch (4) x sequence-parallel (2).
Core c handles batch b=c//2, token half h=c%2 (512 tokens).
Per block each core computes q/k/v for its own tokens, all-gathers
k^T and v (bf16) within its pair, then computes attention for its 512
queries over all 1024 keys. All activations are stored feature-major
([feature(part), token(free)]); scores are computed transposed [tk, tq]
so the softmax sum is a ones-matmul partition reduction and no
transposes are needed anywhere. V is computed token-major directly by
swapping matmul operands, with a ones column appended (M=65 matmul) so
the softmax denominator falls out of the PV matmul.
"""

import os
import sys

sys.path.insert(0, "/opt/trn_rl_repo")

import ml_dtypes
import numpy as np

import concourse.bass as bass
import concourse.mybir as mybir
import concourse.tile as tile
from concourse import bacc
from concourse.bass_utils import run_bass_kernel_spmd

F32 = mybir.dt.float32
F16 = mybir.dt.float16
BF16 = mybir.dt.bfloat16
AF = mybir.ActivationFunctionType
MUL = mybir.AluOpType.mult

B, S, D, NH, HD = 4, 1024, 1024, 16, 64
INTER = 2816
T = S // 2              # own tokens per core
DT = D // 128           # 8 d-tiles
IT = INTER // 128       # 22 inter tiles
VF = NH * (HD + 1)      # 1040, v_aug feature width
EPS = 1e-5
KN = T * S              # kT elems (own): 1024 x 512
VN = T * VF             # v_aug elems (own): 512 x 1040
RG = [[0, 1], [2, 3], [4, 5], [6, 7]]

N_LEVEL_CALLS = int(os.environ.get("HRM_LEVEL_CALLS", "6"))

_CACHE = {}


def _rope(nc, sp, ps, out_ap, cos, sin):
    """out = ps*cos + rotate_half(ps)*sin  (partition dim = 2 heads x 64)."""
    t1 = sp.tile([128, 512], F32, tag="rope1", name="rope1")
    t2 = sp.tile([128, 512], F32, tag="rope2", name="rope2")
    nc.vector.tensor_tensor(t1[:], ps[:], cos[:], MUL)
    nc.vector.tensor_tensor(t2[0:32, :], ps[32:64, :], sin[0:32, :], MUL)
    nc.vector.tensor_tensor(t2[32:64, :], ps[0:32, :], sin[32:64, :], MUL)
    nc.vector.tensor_tensor(t2[64:96, :], ps[96:128, :], sin[64:96, :], MUL)
    nc.vector.tensor_tensor(t2[96:128, :], ps[64:96, :], sin[96:128, :], MUL)
    nc.vector.tensor_add(out=out_ap, in0=t1[:], in1=t2[:])


def _rmsnorm(nc, sp, psum, h, hb, ones128, ones1f, eps_ap):
    ss = psum.tile([1, 512], F32, tag="ss", name="ss")
    for dt in range(DT):
        r2 = sp.tile([128, 512], F16, tag="r2", name="r2")
        nc.vector.tensor_tensor(r2[:], h[:, dt, :], h[:, dt, :], MUL)
        nc.tensor.matmul(ss[:], ones128[:], r2[:], start=(dt == 0), stop=(dt == DT - 1))
    s1 = sp.tile([1, 512], F32, tag="s1", name="s1")
    nc.scalar.activation(s1[:], ss[:], AF.Sqrt, bias=eps_ap, scale=1.0 / D)
    rstd = sp.tile([1, 512], F32, tag="rstd", name="rstd")
    nc.vector.reciprocal(rstd[:], s1[:])
    bc = _bcast(nc, sp, psum, rstd, ones1f)
    for dt in range(DT):
        nc.vector.tensor_tensor(h[:, dt, :], h[:, dt, :], bc[:], MUL)
        nc.vector.tensor_copy(out=hb[:, dt, :], in_=h[:, dt, :])


def _bcast(nc, sp, psum, row_f32, ones1f):
    """Broadcast [1,512] f32 across 128 partitions via K=1 matmul."""
    pb = psum.tile([128, 512], F32, tag="bc", bufs=2, name="pb")
    nc.tensor.matmul(pb[:], ones1f[:], row_f32[:], start=True, stop=True)
    bc = sp.tile([128, 512], F32, tag="bcsb", name="bc")
    nc.scalar.copy(bc[:], pb[:])
    return bc


def build_kernel():
    nc = bacc.Bacc("TRN2", target_bir_lowering=False, debug=False, num_devices=8)

    inp = {}
    for nm, shape, dt in [
        ("zL", [D, T], F32), ("zH", [D, T], F32), ("emb", [D, T], F32),
        ("cosT", [128, T], F32), ("sinT", [128, T], F32),
        ("L_wqT", [2, D, D], F16), ("L_wkT", [2, D, D], F16),
        ("L_wvT", [2, D, D], F16), ("L_woT", [2, D, D], F16),
        ("L_guT", [2, D, 2 * INTER], F16), ("L_dnT", [2, INTER, D], F16),
        ("H_wqT", [2, D, D], F16), ("H_wkT", [2, D, D], F16),
        ("H_wvT", [2, D, D], F16), ("H_woT", [2, D, D], F16),
        ("H_guT", [2, D, 2 * INTER], F16), ("H_dnT", [2, INTER, D], F16),
    ]:
        inp[nm] = nc.dram_tensor(nm, shape, dt, kind="ExternalInput")
    out_t = nc.dram_tensor("zH_out", [D, T], F32, kind="ExternalOutput")

    seq = os.environ.get("HRM_SEQ", "")
    if seq:
        level_calls = list(seq)
    else:
        level_calls = (["L", "L", "H"] * 2)[:N_LEVEL_CALLS]

    with tile.TileContext(nc) as tc:
        with (
            tc.tile_pool(name="state", bufs=1) as st,
            tc.tile_pool(name="sp", bufs=2) as sp,
            tc.tile_pool(name="big", bufs=2) as bigp,
            tc.tile_pool(name="w128", bufs=4) as w128p,
            tc.tile_pool(name="w256", bufs=2) as w256p,
            tc.tile_pool(name="wd", bufs=2) as wdp,
            tc.tile_pool(name="pt", bufs=2) as ptp,
            tc.tile_pool(name="psum", bufs=1, space="PSUM") as psum,
            tc.tile_pool(name="dram", bufs=2, space="DRAM") as dram,
        ):
            zL = st.tile([128, DT, T], F32, name="zL_sb")
            zH = st.tile([128, DT, T], F32, name="zH_sb")
            emb = st.tile([128, DT, T], F32, name="emb_sb")
            cos = st.tile([128, T], F32, name="cos_sb")
            sin = st.tile([128, T], F32, name="sin_sb")
            hb = st.tile([128, DT, T], F16, name="hb")
            qT = st.tile([128, DT, T], F16, name="qT")
            kst = st.tile([128, DT, T], F16, name="kst")
            vst = st.tile([128, 4, VF], BF16, name="vst")
            oT = st.tile([128, DT, T], F16, name="oT")
            ones128 = st.tile([128, 1], F16, name="ones128")
            ones1f = st.tile([1, 128], F32, name="ones1f")
            epsc = st.tile([1, 1], F32, name="epsc")

            nc.sync.dma_start(zL[:], inp["zL"].rearrange("(dt p) t -> p dt t", p=128))
            nc.sync.dma_start(zH[:], inp["zH"].rearrange("(dt p) t -> p dt t", p=128))
            nc.sync.dma_start(emb[:], inp["emb"].rearrange("(dt p) t -> p dt t", p=128))
            nc.sync.dma_start(cos[:], inp["cosT"][:])
            nc.sync.dma_start(sin[:], inp["sinT"][:])
            nc.vector.memset(ones128[:], 1.0)
            nc.vector.memset(ones1f[:], 1.0)
            nc.vector.memset(epsc[:], EPS)
            neg8 = st.tile([128, 1], F32, name="neg8")
            nc.vector.memset(neg8[:], -8.0)
            # ones columns of v_aug (written once; data copies avoid them)
            nc.vector.memset(
                vst.rearrange("p tt (h c) -> p tt h c", c=HD + 1)[:, :, :, HD : HD + 1],
                1.0,
            )

            def block(h, wq, wk, wv, wo, gu, dn):
                gin = dram.tile([KN + VN], F16, name="gin")
                gout = dram.tile([2 * (KN + VN)], F16, name="gout")

                # ---- k projection + rope ----
                for ot in range(DT):
                    w = w128p.tile([128, DT, 128], F16, tag="w128", name="wk")
                    nc.sync.dma_start(w[:], wk[:, :, ot * 128 : (ot + 1) * 128])
                    ps = psum.tile([128, 512], F32, tag="mm", bufs=3, name="psk")
                    for dt in range(DT):
                        nc.tensor.matmul(ps[:], w[:, dt, :], hb[:, dt, :],
                                         start=(dt == 0), stop=(dt == DT - 1))
                    _rope(nc, sp, ps, kst[:, ot, :], cos, sin)
                # ---- v projection (token-major) ----
                vsr = vst.rearrange("p tt (hh c) -> p tt hh c", c=HD + 1)
                for oc in range(4):
                    w = w256p.tile([128, DT, 256], F16, tag="w256", name="wv")
                    nc.sync.dma_start(w[:], wv[:, :, oc * 256 : (oc + 1) * 256])
                    for tt in range(4):
                        ps = psum.tile([128, 512], F32, tag="mm", bufs=3, name="psv")[:, 0:256]
                        for dt in range(DT):
                            nc.tensor.matmul(
                                ps[:], hb[:, dt, tt * 128 : (tt + 1) * 128],
                                w[:, dt, :], start=(dt == 0), stop=(dt == DT - 1))
                        nc.vector.tensor_copy(
                            out=vsr[:, tt, oc * 4 : (oc + 1) * 4, 0:HD],
                            in_=ps.rearrange("p (hh c) -> p hh c", c=HD))
                # ---- send + gather ----
                nc.sync.dma_start(
                    gin[0:KN].rearrange("(dt p t) -> p dt t", p=128, t=T), kst[:])
                nc.sync.dma_start(
                    gin[KN:].rearrange("(tt p f) -> p tt f", p=128, f=VF).bitcast(BF16), vst[:])
                nc.gpsimd.collective_compute(
                    "AllGather", mybir.AluOpType.bypass, replica_groups=RG,
                    ins=[gin.opt()], outs=[gout.opt()])
                # ---- q projection + rope (overlaps gather) ----
                for ot in range(DT):
                    w = w128p.tile([128, DT, 128], F16, tag="w128", name="wq")
                    nc.sync.dma_start(w[:], wq[:, :, ot * 128 : (ot + 1) * 128])
                    ps = psum.tile([128, 512], F32, tag="mm", bufs=3, name="psq")
                    for dt in range(DT):
                        nc.tensor.matmul(ps[:], w[:, dt, :], hb[:, dt, :],
                                         start=(dt == 0), stop=(dt == DT - 1))
                    _rope(nc, sp, ps, qT[:, ot, :], cos, sin)
                # ---- load gathered k/v ----
                kTf = bigp.tile([128, DT, S], F16, tag="big", name="kTf")
                vf = bigp.tile([128, DT, VF], BF16, tag="big", name="vf")
                for r in range(2):
                    base = r * (KN + VN)
                    nc.sync.dma_start(
                        kTf[:, :, r * T : (r + 1) * T],
                        gout[base : base + KN].rearrange(
                            "(dt p t) -> p dt t", p=128, t=T))
                    nc.sync.dma_start(
                        vf[:, 4 * r : 4 * r + 4, :],
                        gout[base + KN : base + KN + VN].rearrange(
                            "(tt p f) -> p tt f", p=128, f=VF).bitcast(BF16))
                # ---- attention, head pairs: scores A/B interleaved over
                # row groups (concurrent on PE), normalize deferred so PE
                # never waits on the DVE/ACT recip chain ----
                for ot in range(DT):
                    pts = []
                    pvs = []
                    for sub in range(2):
                        bp = sub * 64
                        pt = ptp.tile([128, DT, 512], BF16, tag="pt", bufs=2, name="pt")
                        pts.append(pt)
                    for kt in range(DT):
                        for sub in range(2):
                            bp = sub * 64
                            pss = psum.tile([128, 512], F32, tag="mm", bufs=3,
                                            name="pss")
                            nc.tensor.matmul(
                                pss[:],
                                kTf[bp : bp + 64, ot, kt * 128 : (kt + 1) * 128],
                                qT[bp : bp + 64, ot, :],
                                start=True, stop=True, tile_position=(bp, 0))
                            nc.scalar.activation(pts[sub][:, kt, :], pss[:],
                                                 AF.Exp, scale=0.125)
                    for sub in range(2):
                        hh = ot * 2 + sub
                        pv = psum.tile([128, 512], F32, tag="pv", bufs=2, name="pv")
                        for kt in range(DT):
                            nc.tensor.matmul(
                                pv[0 : HD + 1, :],
                                vf[:, kt, hh * (HD + 1) : (hh + 1) * (HD + 1)],
                                pts[sub][:, kt, :],
                                start=(kt == 0), stop=(kt == DT - 1))
                        pvs.append(pv)
                    for sub in range(2):
                        hh = ot * 2 + sub
                        bp = sub * 64
                        pv = pvs[sub]
                        recip = sp.tile([1, 512], F32, tag="recip", name="recip")
                        nc.vector.reciprocal(recip[:], pv[HD : HD + 1, :])
                        bc = _bcast(nc, sp, psum, recip, ones1f)
                        nc.vector.tensor_tensor(
                            oT[bp : bp + 64, ot, :], pv[0:HD, :], bc[0:HD, :], MUL)
                # ---- o projection + residual ----
                for dt2 in range(DT):
                    w = w128p.tile([128, DT, 128], F16, tag="w128", name="wo")
                    nc.sync.dma_start(w[:], wo[:, :, dt2 * 128 : (dt2 + 1) * 128])
                    ps = psum.tile([128, 512], F32, tag="mm", bufs=3, name="pso")
                    for et in range(DT):
                        nc.tensor.matmul(ps[:], w[:, et, :], oT[:, et, :],
                                         start=(et == 0), stop=(et == DT - 1))
                    nc.vector.tensor_add(out=h[:, dt2, :], in0=h[:, dt2, :], in1=ps[:])
                _rmsnorm(nc, sp, psum, h, hb, ones128, ones1f, epsc[:])
                # ---- MLP ----
                act = bigp.tile([128, IT, 512], F16, tag="big", name="act")
                for it in range(IT):
                    wg = w128p.tile([128, DT, 128], F16, tag="w128", name="wg")
                    wu = w128p.tile([128, DT, 128], F16, tag="w128", name="wu")
                    nc.sync.dma_start(wg[:], gu[:, :, it * 128 : (it + 1) * 128])
                    nc.sync.dma_start(
                        wu[:], gu[:, :, INTER + it * 128 : INTER + (it + 1) * 128])
                    psg = psum.tile([128, 512], F32, tag="mm", bufs=3, name="psg")
                    psu = psum.tile([128, 512], F32, tag="mm", bufs=3, name="psu")
                    for dt in range(DT):
                        nc.tensor.matmul(psg[:], wg[:, dt, :], hb[:, dt, :],
                                         start=(dt == 0), stop=(dt == DT - 1))
                    for dt in range(DT):
                        nc.tensor.matmul(psu[:], wu[:, dt, :], hb[:, dt, :],
                                         start=(dt == 0), stop=(dt == DT - 1))
                    sg = sp.tile([128, 512], F16, tag="sg", name="sg")
                    nc.scalar.activation(sg[:], psg[:], AF.Silu)
                    nc.vector.tensor_tensor(act[:, it, :], psu[:], sg[:], MUL)
                for dt2 in range(DT):
                    w = wdp.tile([128, IT, 128], F16, tag="wd", name="wdn")
                    nc.sync.dma_start(w[:], dn[:, :, dt2 * 128 : (dt2 + 1) * 128])
                    ps = psum.tile([128, 512], F32, tag="mm", bufs=3, name="psd")
                    for it in range(IT):
                        nc.tensor.matmul(ps[:], w[:, it, :], act[:, it, :],
                                         start=(it == 0), stop=(it == IT - 1))
                    nc.vector.tensor_add(out=h[:, dt2, :], in0=h[:, dt2, :], in1=ps[:])
                _rmsnorm(nc, sp, psum, h, hb, ones128, ones1f, epsc[:])

            def wrearr(ap):  # [K, M] -> [128, K//128, M] tiled view
                return ap.rearrange("(kt p) m -> p kt m", p=128)

            for lvl in level_calls:
                if lvl == "L":
                    h = zL
                    for dt in range(DT):
                        nc.vector.tensor_add(out=h[:, dt, :], in0=h[:, dt, :],
                                             in1=zH[:, dt, :])
                        nc.vector.tensor_add(out=h[:, dt, :], in0=h[:, dt, :],
                                             in1=emb[:, dt, :])
                    pre = "L"
                else:
                    h = zH
                    for dt in range(DT):
                        nc.vector.tensor_add(out=h[:, dt, :], in0=h[:, dt, :],
                                             in1=zL[:, dt, :])
                    pre = "H"
                for dt in range(DT):
                    nc.vector.tensor_copy(out=hb[:, dt, :], in_=h[:, dt, :])
                for i in range(2):
                    block(
                        h,
                        wrearr(inp[f"{pre}_wqT"][i]), wrearr(inp[f"{pre}_wkT"][i]),
                        wrearr(inp[f"{pre}_wvT"][i]), wrearr(inp[f"{pre}_woT"][i]),
                        wrearr(inp[f"{pre}_guT"][i]), wrearr(inp[f"{pre}_dnT"][i]),
                    )

            nc.sync.dma_start(
                out_t.rearrange("(dt p) t -> p dt t", p=128), zH[:])

    nc.compile()
    return nc


def _prep_weights(inputs):
    bf = np.float16
    w = {}
    for pre in ("L", "H"):
        for nm, src in [("wqT", "wq"), ("wkT", "wk"), ("wvT", "wv"), ("woT", "wo"),
                        ("guT", "gu"), ("dnT", "dn")]:
            a = np.asarray(inputs[f"{pre}_{src}"])
            w[f"{pre}_{nm}"] = np.ascontiguousarray(
                a.transpose(0, 2, 1)).astype(bf)
    cos = np.asarray(inputs["cos"])  # [S, 64]
    sin = np.asarray(inputs["sin"])
    cosT = np.tile(cos.T, (2, 1)).astype(np.float32)          # [128, S]
    sinT_s = sin.T.copy()
    sinT_s[:32] *= -1.0
    sinT = np.tile(sinT_s, (2, 1)).astype(np.float32)          # [128, S]
    return w, cosT, sinT


def kernel(**inputs):
    key = "nc"
    if key not in _CACHE:
        _CACHE[key] = build_kernel()
    nc = _CACHE[key]

    w, cosT, sinT = _prep_weights(inputs)
    zL = np.asarray(inputs["z_L"], np.float32)
    zH = np.asarray(inputs["z_H"], np.float32)
    emb = np.asarray(inputs["input_emb"], np.float32)

    in_maps = []
    for c in range(8):
        b, half = c // 2, c % 2
        sl = slice(half * T, (half + 1) * T)
        m = {
            "zL": np.ascontiguousarray(zL[b].T[:, sl]),
            "zH": np.ascontiguousarray(zH[b].T[:, sl]),
            "emb": np.ascontiguousarray(emb[b].T[:, sl]),
            "cosT": np.ascontiguousarray(cosT[:, sl]),
            "sinT": np.ascontiguousarray(sinT[:, sl]),
        }
        m.update(w)
        in_maps.append(m)

    trace = os.environ.get("HRM_TRACE", "0") == "1"
    res = run_bass_kernel_spmd(nc, in_maps, core_ids=list(range(8)), trace=trace)
    _CACHE["last_result"] = res

    out = np.empty((B, S, D), np.float32)
    for c in range(8):
        b, half = c // 2, c % 2
        out[b, half * T : (half + 1) * T, :] = res.results[c]["zH_out"].T
    return out


if __name__ == "__main__":
    rng = np.random.default_rng(0)
    ins = {
        "z_H": rng.standard_normal((B, S, D), np.float32),
        "z_L": rng.standard_normal((B, S, D), np.float32),
        "input_emb": rng.standard_normal((B, S, D), np.float32),
    }
    sd = 1.0 / np.sqrt(D)
    si = 1.0 / np.sqrt(INTER)
    for pre in ("L", "H"):
        for nm, shape, s in [("wq", (2, D, D), sd), ("wk", (2, D, D), sd),
                             ("wv", (2, D, D), sd), ("wo", (2, D, D), sd),
                             ("gu", (2, 2 * INTER, D), sd), ("dn", (2, D, INTER), si)]:
            ins[f"{pre}_{nm}"] = rng.standard_normal(shape, np.float32) * s
    inv = 1.0 / (10000.0 ** (np.arange(0, HD, 2, np.float32) / HD))
    fr = np.outer(np.arange(S, np.float32), inv)
    e = np.concatenate([fr, fr], -1)
    ins["cos"], ins["sin"] = np.cos(e).astype(np.float32), np.sin(e).astype(np.float32)
    out = kernel(**ins)
    print("out", out.shape, out.dtype, np.abs(out).mean())



# revision 9
# speedup vs baseline: 1.0585x; 1.0585x over previous
"""HRM dense-transformer kernel for 8 trn2 NeuronCores.

Sharding: data-parallel over batch (4) x sequence-parallel (2).
Core c handles batch b=c//2, token half h=c%2 (512 tokens).
Per block each core computes q/k/v for its own tokens, all-gathers
k^T and v (bf16) within its pair, then computes attention for its 512
queries over all 1024 keys. All activations are stored feature-major
([feature(part), token(free)]); scores are computed transposed [tk, tq]
so the softmax sum is a ones-matmul partition reduction and no
transposes are needed anywhere. V is computed token-major directly by
swapping matmul operands, with a ones column appended (M=65 matmul) so
the softmax denominator falls out of the PV matmul.
"""

import os
import sys

sys.path.insert(0, "/opt/trn_rl_repo")

import ml_dtypes
import numpy as np

import concourse.bass as bass
import concourse.mybir as mybir
import concourse.tile as tile
from concourse import bacc
from concourse.bass_utils import run_bass_kernel_spmd

F32 = mybir.dt.float32
F16 = mybir.dt.float16
BF16 = mybir.dt.bfloat16
AF = mybir.ActivationFunctionType
MUL = mybir.AluOpType.mult

B, S, D, NH, HD = 4, 1024, 1024, 16, 64
INTER = 2816
T = S // 2              # own tokens per core
DT = D // 128           # 8 d-tiles
IT = INTER // 128       # 22 inter tiles
VF = NH * (HD + 1)      # 1040, v_aug feature width
EPS = 1e-5
KN = T * S              # kT elems (own): 1024 x 512
VN = T * VF             # v_aug elems (own): 512 x 1040
RG = [[0, 1], [2, 3], [4, 5], [6, 7]]

N_LEVEL_CALLS = int(os.environ.get("HRM_LEVEL_CALLS", "6"))

_CACHE = {}


def _rope(nc, sp, ps, out_ap, cos, sin):
    """out = ps*cos + rotate_half(ps)*sin  (partition dim = 2 heads x 64)."""
    t1 = sp.tile([128, 512], F32, tag="rope1", name="rope1")
    t2 = sp.tile([128, 512], F32, tag="rope2", name="rope2")
    nc.vector.tensor_tensor(t1[:], ps[:], cos[:], MUL)
    nc.vector.tensor_tensor(t2[0:32, :], ps[32:64, :], sin[0:32, :], MUL)
    nc.vector.tensor_tensor(t2[32:64, :], ps[0:32, :], sin[32:64, :], MUL)
    nc.vector.tensor_tensor(t2[64:96, :], ps[96:128, :], sin[64:96, :], MUL)
    nc.vector.tensor_tensor(t2[96:128, :], ps[64:96, :], sin[96:128, :], MUL)
    nc.vector.tensor_add(out=out_ap, in0=t1[:], in1=t2[:])


def _rmsnorm(nc, sp, psum, h, hb, ones128, ones1f, eps_ap):
    ss = psum.tile([1, 512], F32, tag="ss", name="ss")
    for dt in range(DT):
        r2 = sp.tile([128, 512], F16, tag="r2", name="r2")
        nc.vector.tensor_tensor(r2[:], h[:, dt, :], h[:, dt, :], MUL)
        nc.tensor.matmul(ss[:], ones128[:], r2[:], start=(dt == 0), stop=(dt == DT - 1))
    s1 = sp.tile([1, 512], F32, tag="s1", name="s1")
    nc.scalar.activation(s1[:], ss[:], AF.Sqrt, bias=eps_ap, scale=1.0 / D)
    rstd = sp.tile([1, 512], F32, tag="rstd", name="rstd")
    nc.vector.reciprocal(rstd[:], s1[:])
    bc = _bcast(nc, sp, psum, rstd, ones1f)
    for dt in range(DT):
        nc.vector.tensor_tensor(h[:, dt, :], h[:, dt, :], bc[:], MUL)
        nc.vector.tensor_copy(out=hb[:, dt, :], in_=h[:, dt, :])


def _bcast(nc, sp, psum, row_f32, ones1f):
    """Broadcast [1,512] f32 across 128 partitions via K=1 matmul."""
    pb = psum.tile([128, 512], F32, tag="bc", bufs=2, name="pb")
    nc.tensor.matmul(pb[:], ones1f[:], row_f32[:], start=True, stop=True)
    bc = sp.tile([128, 512], F32, tag="bcsb", name="bc")
    nc.scalar.copy(bc[:], pb[:])
    return bc


def build_kernel():
    nc = bacc.Bacc("TRN2", target_bir_lowering=False, debug=False, num_devices=8)

    inp = {}
    for nm, shape, dt in [
        ("zL", [D, T], F32), ("zH", [D, T], F32), ("emb", [D, T], F32),
        ("cosT", [128, T], F32), ("sinT", [128, T], F32),
        ("L_wqT", [2, D, D], F16), ("L_wkT", [2, D, D], F16),
        ("L_wvT", [2, D, D], F16), ("L_woT", [2, D, D], F16),
        ("L_guT", [2, D, 2 * INTER], F16), ("L_dnT", [2, INTER, D], F16),
        ("H_wqT", [2, D, D], F16), ("H_wkT", [2, D, D], F16),
        ("H_wvT", [2, D, D], F16), ("H_woT", [2, D, D], F16),
        ("H_guT", [2, D, 2 * INTER], F16), ("H_dnT", [2, INTER, D], F16),
    ]:
        inp[nm] = nc.dram_tensor(nm, shape, dt, kind="ExternalInput")
    out_t = nc.dram_tensor("zH_out", [D, T], F32, kind="ExternalOutput")

    seq = os.environ.get("HRM_SEQ", "")
    if seq:
        level_calls = list(seq)
    else:
        level_calls = (["L", "L", "H"] * 2)[:N_LEVEL_CALLS]

    with tile.TileContext(nc) as tc:
        with (
            tc.tile_pool(name="state", bufs=1) as st,
            tc.tile_pool(name="sp", bufs=2) as sp,
            tc.tile_pool(name="big", bufs=2) as bigp,
            tc.tile_pool(name="w128", bufs=4) as w128p,
            tc.tile_pool(name="w256", bufs=2) as w256p,
            tc.tile_pool(name="wd", bufs=2) as wdp,
            tc.tile_pool(name="pt", bufs=2) as ptp,
            tc.tile_pool(name="psum", bufs=1, space="PSUM") as psum,
            tc.tile_pool(name="dram", bufs=2, space="DRAM") as dram,
        ):
            zL = st.tile([128, DT, T], F32, name="zL_sb")
            zH = st.tile([128, DT, T], F32, name="zH_sb")
            emb = st.tile([128, DT, T], F32, name="emb_sb")
            cos = st.tile([128, T], F32, name="cos_sb")
            sin = st.tile([128, T], F32, name="sin_sb")
            hb = st.tile([128, DT, T], F16, name="hb")
            qT = st.tile([128, DT, T], F16, name="qT")
            kst = st.tile([128, DT, T], F16, name="kst")
            vst = st.tile([128, 4, VF], BF16, name="vst")
            oT = st.tile([128, DT, T], F16, name="oT")
            ones128 = st.tile([128, 1], F16, name="ones128")
            ones1f = st.tile([1, 128], F32, name="ones1f")
            epsc = st.tile([1, 1], F32, name="epsc")

            nc.sync.dma_start(zL[:], inp["zL"].rearrange("(dt p) t -> p dt t", p=128))
            nc.sync.dma_start(zH[:], inp["zH"].rearrange("(dt p) t -> p dt t", p=128))
            nc.sync.dma_start(emb[:], inp["emb"].rearrange("(dt p) t -> p dt t", p=128))
            nc.sync.dma_start(cos[:], inp["cosT"][:])
            nc.sync.dma_start(sin[:], inp["sinT"][:])
            nc.vector.memset(ones128[:], 1.0)
            nc.vector.memset(ones1f[:], 1.0)
            nc.vector.memset(epsc[:], EPS)
            neg8 = st.tile([128, 1], F32, name="neg8")
            nc.vector.memset(neg8[:], -8.0)
            # ones columns of v_aug (written once; data copies avoid them)
            nc.vector.memset(
                vst.rearrange("p tt (h c) -> p tt h c", c=HD + 1)[:, :, :, HD : HD + 1],
                1.0,
            )

            def block(h, wq, wk, wv, wo, gu, dn):
                gin_k = dram.tile([KN], F16, tag="gin_k", name="gin_k")
                gout_k = dram.tile([2 * KN], F16, tag="gout_k", name="gout_k")
                gin_v = dram.tile([VN], BF16, tag="gin_v", name="gin_v")
                gout_v = dram.tile([2 * VN], BF16, tag="gout_v", name="gout_v")

                # ---- k projection + rope ----
                for ot in range(DT):
                    w = w128p.tile([128, DT, 128], F16, tag="w128", name="wk")
                    nc.sync.dma_start(w[:], wk[:, :, ot * 128 : (ot + 1) * 128])
                    ps = psum.tile([128, 512], F32, tag="mm", bufs=3, name="psk")
                    for dt in range(DT):
                        nc.tensor.matmul(ps[:], w[:, dt, :], hb[:, dt, :],
                                         start=(dt == 0), stop=(dt == DT - 1))
                    _rope(nc, sp, ps, kst[:, ot, :], cos, sin)
                # ---- send + gather k (overlaps v/q projection) ----
                nc.sync.dma_start(
                    gin_k[:].rearrange("(dt p t) -> p dt t", p=128, t=T), kst[:])
                nc.gpsimd.collective_compute(
                    "AllGather", mybir.AluOpType.bypass, replica_groups=RG,
                    ins=[gin_k.opt()], outs=[gout_k.opt()])
                # ---- v projection (token-major) ----
                vsr = vst.rearrange("p tt (hh c) -> p tt hh c", c=HD + 1)
                for oc in range(4):
                    w = w256p.tile([128, DT, 256], F16, tag="w256", name="wv")
                    nc.sync.dma_start(w[:], wv[:, :, oc * 256 : (oc + 1) * 256])
                    for tt in range(4):
                        ps = psum.tile([128, 512], F32, tag="mm", bufs=3, name="psv")[:, 0:256]
                        for dt in range(DT):
                            nc.tensor.matmul(
                                ps[:], hb[:, dt, tt * 128 : (tt + 1) * 128],
                                w[:, dt, :], start=(dt == 0), stop=(dt == DT - 1))
                        nc.vector.tensor_copy(
                            out=vsr[:, tt, oc * 4 : (oc + 1) * 4, 0:HD],
                            in_=ps.rearrange("p (hh c) -> p hh c", c=HD))
                # ---- send + gather v (k already in flight) ----
                nc.sync.dma_start(
                    gin_v[:].rearrange("(tt p f) -> p tt f", p=128, f=VF), vst[:])
                nc.gpsimd.collective_compute(
                    "AllGather", mybir.AluOpType.bypass, replica_groups=RG,
                    ins=[gin_v.opt()], outs=[gout_v.opt()])
                # ---- q projection + rope (overlaps gather) ----
                for ot in range(DT):
                    w = w128p.tile([128, DT, 128], F16, tag="w128", name="wq")
                    nc.sync.dma_start(w[:], wq[:, :, ot * 128 : (ot + 1) * 128])
                    ps = psum.tile([128, 512], F32, tag="mm", bufs=3, name="psq")
                    for dt in range(DT):
                        nc.tensor.matmul(ps[:], w[:, dt, :], hb[:, dt, :],
                                         start=(dt == 0), stop=(dt == DT - 1))
                    _rope(nc, sp, ps, qT[:, ot, :], cos, sin)
                # ---- load gathered k/v ----
                kTf = bigp.tile([128, DT, S], F16, tag="big", name="kTf")
                vf = bigp.tile([128, DT, VF], BF16, tag="big", name="vf")
                for r in range(2):
                    nc.sync.dma_start(
                        kTf[:, :, r * T : (r + 1) * T],
                        gout_k[r * KN : (r + 1) * KN].rearrange(
                            "(dt p t) -> p dt t", p=128, t=T))
                    nc.sync.dma_start(
                        vf[:, 4 * r : 4 * r + 4, :],
                        gout_v[r * VN : (r + 1) * VN].rearrange(
                            "(tt p f) -> p tt f", p=128, f=VF))
                # ---- attention, head pairs: scores A/B interleaved over
                # row groups (concurrent on PE), normalize deferred so PE
                # never waits on the DVE/ACT recip chain ----
                for ot in range(DT):
                    pts = []
                    pvs = []
                    for sub in range(2):
                        bp = sub * 64
                        pt = ptp.tile([128, DT, 512], BF16, tag="pt", bufs=2, name="pt")
                        pts.append(pt)
                    for kt in range(DT):
                        for sub in range(2):
                            bp = sub * 64
                            pss = psum.tile([128, 512], F32, tag="mm", bufs=3,
                                            name="pss")
                            nc.tensor.matmul(
                                pss[:],
                                kTf[bp : bp + 64, ot, kt * 128 : (kt + 1) * 128],
                                qT[bp : bp + 64, ot, :],
                                start=True, stop=True, tile_position=(bp, 0))
                            nc.scalar.activation(pts[sub][:, kt, :], pss[:],
                                                 AF.Exp, scale=0.125)
                    for sub in range(2):
                        hh = ot * 2 + sub
                        pv = psum.tile([128, 512], F32, tag="pv", bufs=2, name="pv")
                        for kt in range(DT):
                            nc.tensor.matmul(
                                pv[0 : HD + 1, :],
                                vf[:, kt, hh * (HD + 1) : (hh + 1) * (HD + 1)],
                                pts[sub][:, kt, :],
                                start=(kt == 0), stop=(kt == DT - 1))
                        pvs.append(pv)
                    for sub in range(2):
                        hh = ot * 2 + sub
                        bp = sub * 64
                        pv = pvs[sub]
                        recip = sp.tile([1, 512], F32, tag="recip", name="recip")
                        nc.vector.reciprocal(recip[:], pv[HD : HD + 1, :])
                        bc = _bcast(nc, sp, psum, recip, ones1f)
                        nc.vector.tensor_tensor(
                            oT[bp : bp + 64, ot, :], pv[0:HD, :], bc[0:HD, :], MUL)
                # ---- o projection + residual ----
                for dt2 in range(DT):
                    w = w128p.tile([128, DT, 128], F16, tag="w128", name="wo")
                    nc.sync.dma_start(w[:], wo[:, :, dt2 * 128 : (dt2 + 1) * 128])
                    ps = psum.tile([128, 512], F32, tag="mm", bufs=3, name="pso")
                    for et in range(DT):
                        nc.tensor.matmul(ps[:], w[:, et, :], oT[:, et, :],
                                         start=(et == 0), stop=(et == DT - 1))
                    nc.vector.tensor_add(out=h[:, dt2, :], in0=h[:, dt2, :], in1=ps[:])
                _rmsnorm(nc, sp, psum, h, hb, ones128, ones1f, epsc[:])
                # ---- MLP ----
                act = bigp.tile([128, IT, 512], F16, tag="big", name="act")
                for it in range(IT):
                    wg = w128p.tile([128, DT, 128], F16, tag="w128", name="wg")
                    wu = w128p.tile([128, DT, 128], F16, tag="w128", name="wu")
                    nc.sync.dma_start(wg[:], gu[:, :, it * 128 : (it + 1) * 128])
                    nc.sync.dma_start(
                        wu[:], gu[:, :, INTER + it * 128 : INTER + (it + 1) * 128])
                    psg = psum.tile([128, 512], F32, tag="mm", bufs=3, name="psg")
                    psu = psum.tile([128, 512], F32, tag="mm", bufs=3, name="psu")
                    for dt in range(DT):
                        nc.tensor.matmul(psg[:], wg[:, dt, :], hb[:, dt, :],
                                         start=(dt == 0), stop=(dt == DT - 1))
                    for dt in range(DT):
                        nc.tensor.matmul(psu[:], wu[:, dt, :], hb[:, dt, :],
                                         start=(dt == 0), stop=(dt == DT - 1))
                    sg = sp.tile([128, 512], F16, tag="sg", name="sg")
                    nc.scalar.activation(sg[:], psg[:], AF.Silu)
                    nc.vector.tensor_tensor(act[:, it, :], psu[:], sg[:], MUL)
                for dt2 in range(DT):
                    w = wdp.tile([128, IT, 128], F16, tag="wd", name="wdn")
                    nc.sync.dma_start(w[:], dn[:, :, dt2 * 128 : (dt2 + 1) * 128])
                    ps = psum.tile([128, 512], F32, tag="mm", bufs=3, name="psd")
                    for it in range(IT):
                        nc.tensor.matmul(ps[:], w[:, it, :], act[:, it, :],
                                         start=(it == 0), stop=(it == IT - 1))
                    nc.vector.tensor_add(out=h[:, dt2, :], in0=h[:, dt2, :], in1=ps[:])
                _rmsnorm(nc, sp, psum, h, hb, ones128, ones1f, epsc[:])

            def wrearr(ap):  # [K, M] -> [128, K//128, M] tiled view
                return ap.rearrange("(kt p) m -> p kt m", p=128)

            for lvl in level_calls:
                if lvl == "L":
                    h = zL
                    for dt in range(DT):
                        nc.vector.tensor_add(out=h[:, dt, :], in0=h[:, dt, :],
                                             in1=zH[:, dt, :])
                        nc.vector.tensor_add(out=h[:, dt, :], in0=h[:, dt, :],
                                             in1=emb[:, dt, :])
                    pre = "L"
                else:
                    h = zH
                    for dt in range(DT):
                        nc.vector.tensor_add(out=h[:, dt, :], in0=h[:, dt, :],
                                             in1=zL[:, dt, :])
                    pre = "H"
                for dt in range(DT):
                    nc.vector.tensor_copy(out=hb[:, dt, :], in_=h[:, dt, :])
                for i in range(2):
                    block(
                        h,
                        wrearr(inp[f"{pre}_wqT"][i]), wrearr(inp[f"{pre}_wkT"][i]),
                        wrearr(inp[f"{pre}_wvT"][i]), wrearr(inp[f"{pre}_woT"][i]),
                        wrearr(inp[f"{pre}_guT"][i]), wrearr(inp[f"{pre}_dnT"][i]),
                    )

            nc.sync.dma_start(
                out_t.rearrange("(dt p) t -> p dt t", p=128), zH[:])

    nc.compile()
    return nc


def _prep_weights(inputs):
    bf = np.float16
    w = {}
    for pre in ("L", "H"):
        for nm, src in [("wqT", "wq"), ("wkT", "wk"), ("wvT", "wv"), ("woT", "wo"),
                        ("guT", "gu"), ("dnT", "dn")]:
            a = np.asarray(inputs[f"{pre}_{src}"])
            w[f"{pre}_{nm}"] = np.ascontiguousarray(
                a.transpose(0, 2, 1)).astype(bf)
    cos = np.asarray(inputs["cos"])  # [S, 64]
    sin = np.asarray(inputs["sin"])
    cosT = np.tile(cos.T, (2, 1)).astype(np.float32)          # [128, S]
    sinT_s = sin.T.copy()
    sinT_s[:32] *= -1.0
    sinT = np.tile(sinT_s, (2, 1)).astype(np.float32)          # [128, S]
    return w, cosT, sinT


def kernel(**inputs):
    key = "nc"
    if key not in _CACHE:
        _CACHE[key] = build_kernel()
    nc = _CACHE[key]

    w, cosT, sinT = _prep_weights(inputs)
    zL = np.asarray(inputs["z_L"], np.float32)
    zH = np.asarray(inputs["z_H"], np.float32)
    emb = np.asarray(inputs["input_emb"], np.float32)

    in_maps = []
    for c in range(8):
        b, half = c // 2, c % 2
        sl = slice(half * T, (half + 1) * T)
        m = {
            "zL": np.ascontiguousarray(zL[b].T[:, sl]),
            "zH": np.ascontiguousarray(zH[b].T[:, sl]),
            "emb": np.ascontiguousarray(emb[b].T[:, sl]),
            "cosT": np.ascontiguousarray(cosT[:, sl]),
            "sinT": np.ascontiguousarray(sinT[:, sl]),
        }
        m.update(w)
        in_maps.append(m)

    trace = os.environ.get("HRM_TRACE", "0") == "1"
    res = run_bass_kernel_spmd(nc, in_maps, core_ids=list(range(8)), trace=trace)
    _CACHE["last_result"] = res

    out = np.empty((B, S, D), np.float32)
    for c in range(8):
        b, half = c // 2, c % 2
        out[b, half * T : (half + 1) * T, :] = res.results[c]["zH_out"].T
    return out


if __name__ == "__main__":
    rng = np.random.default_rng(0)
    ins = {
        "z_H": rng.standard_normal((B, S, D), np.float32),
        "z_L": rng.standard_normal((B, S, D), np.float32),
        "input_emb": rng.standard_normal((B, S, D), np.float32),
    }
    sd = 1.0 / np.sqrt(D)
    si = 1.0 / np.sqrt(INTER)
    for pre in ("L", "H"):
        for nm, shape, s in [("wq", (2, D, D), sd), ("wk", (2, D, D), sd),
                             ("wv", (2, D, D), sd), ("wo", (2, D, D), sd),
                             ("gu", (2, 2 * INTER, D), sd), ("dn", (2, D, INTER), si)]:
            ins[f"{pre}_{nm}"] = rng.standard_normal(shape, np.float32) * s
    inv = 1.0 / (10000.0 ** (np.arange(0, HD, 2, np.float32) / HD))
    fr = np.outer(np.arange(S, np.float32), inv)
    e = np.concatenate([fr, fr], -1)
    ins["cos"], ins["sin"] = np.cos(e).astype(np.float32), np.sin(e).astype(np.float32)
    out = kernel(**ins)
    print("out", out.shape, out.dtype, np.abs(out).mean())

